# revision 45
# baseline (speedup 1.0000x reference)
"""Trainium2 Bass kernel for nn_AxisAttention (sparse_attention).

Math: the reference applies softmax over a size-1 axis, so every attention
weight is exactly 1.0 and the module collapses algebraically:

    v       = g @ Wv + bv                      # [N, N, D]
    row_att = N * v.transpose(1, 0, 2)         # sum_i of i-independent rows
    col_att = N * v
    out     = g + N*(v + v^T) + ...            # ^T swaps the first two axes
            = g + N*((g + g^T) @ Wv) + 2*N*bv

So one matmul over h = g + g^T suffices; q/k are dead code.

Sharding: the (i, j) grid is split into 32x32 blocks (12x12 of them).
A block B=(bi,bj) is paired with its transpose partner B'=(bj,bi).  With
h_B = g_B + g_B'^T(local) and u_B = h_B @ (N*Wv):

    out_B  = g_B  + u_B  (+ 2N*bv)
    out_B' = g_B' + u_B^T(local) (+ 2N*bv)      since h_B' = h_B^T(local)

so one matmul pass produces BOTH output blocks -> half the FLOPs and every
g/out byte crosses HBM exactly once.  66 pair-units + 12 diagonal units
(+2 dummies) = 80 units, 10 per core on 8 cores -- a uniform SPMD program.

On-device per unit: DMA X=g_B (straight rows) and Yp=g_B' (transpose-permuted
rows, contiguous 2KB runs), DVE h=X+Yp, PE-transpose h tiles (fp32 can't DMA
-transpose), matmul hT-tiles (stationary) against N*Wv (moving), DVE residual
adds, DMA out (straight + permuted APs).
"""

import os
from contextlib import ExitStack

import numpy as np

import concourse.bass as bass
import concourse.bacc as bacc
import concourse.mybir as mybir
import concourse.tile as tile
from concourse.bass_utils import run_bass_kernel_spmd
from concourse.masks import make_identity

# Problem constants (hardcoded per the harness contract).
N = 384          # grid side
D = 512          # feature dim (= contraction dim of Wv)
W = 32           # block side
GB = N // W      # 12 blocks per grid side
NCORES = 8
TP = 128         # SBUF/PSUM partitions per tile
I2 = TP // W     # 4 block-rows per 128-partition tile
NT = (W * W) // TP   # 8 f-tiles per block (f = i*W + j)
KC = D // TP     # 4 contraction chunks

F32 = mybir.dt.float32
F32R = mybir.dt.float32r

# "f32"  -> exact fp32 matmuls (4 PE passes per row, safest numerics)
# "f32r" -> fp32 data truncated to ~fp22 in the PE (1 pass, ~4x faster,
#           rel err ~1e-4); numerically validated against the reference.
MM_MODE = os.environ.get("AXATTN_MM_MODE", "f32")

LAST_RESULTS = None  # BassKernelResults of the most recent run (for test.py)


def _assignment():
    """80 uniform units over 8 cores: unit = (A, B) block-index pair or None."""
    pairs = [((a, b), (b, a)) for a in range(GB) for b in range(a + 1, GB)]
    diags = [((m, m), (m, m)) for m in range(GB)]
    units = pairs + diags                       # 66 + 12 = 78
    per_core = -(-len(units) // NCORES)         # 10
    units += [None] * (NCORES * per_core - len(units))
    return [units[c * per_core:(c + 1) * per_core] for c in range(NCORES)], per_core


DEFAULT_TUNE = {
    "bufs_xy": 3,     # X/Yp input staging buffers
    "bufs_o": 2,      # O1/O2 output staging buffers
    "bufs_h": 3,
    "bufs_ht": 3,
    "bufs_tps": 3,    # transpose PSUM banks
    "bufs_ups": 2,    # matmul-accum PSUM banks
    "o2_engine": "vector",  # engine for the O2 residual add
    "store_engine": "gpsimd",  # out-DMA queue; separate from the load queue
                               # (SP) to avoid head-of-line blocking: stores
                               # wait on compute and would stall later
                               # prefetch loads issued on the same sequencer
}


def _build(n_units: int, with_bias: bool, mm_mode: str, split_dma: bool = True,
           repeat: int = 1, tune: dict | None = None):
    """Build the per-core Bass/Tile program (same program on all 8 cores).

    repeat > 1 wraps the whole unit loop in a device-side For_i that redoes
    the identical work `repeat` times (idempotent) — used only for timing:
    slope between two repeat values isolates pure device time from RPC.
    """
    tn = dict(DEFAULT_TUNE)
    if tune:
        tn.update(tune)
    nc = bacc.Bacc(trn_type="TRN2", target_bir_lowering=False, debug=False)

    g_in = nc.dram_tensor("g_in", [n_units, 2, W, W, D], F32,
                          kind="ExternalInput").ap()
    wv = nc.dram_tensor("wv", [D, D], F32, kind="ExternalInput").ap()
    out = nc.dram_tensor("out_blocks", [n_units, 2, W, W, D], F32,
                         kind="ExternalOutput").ap()
    if with_bias:
        bv = nc.dram_tensor("bv", [1, D], F32, kind="ExternalInput").ap()

    # float32r: PE reads fp32 bits truncated to ~fp22 and runs 1 pass/row
    # instead of fp32's 4 (4x matmul throughput, ~1.6e-4 rel err measured).
    # The BIR verifier requires every PE input to be *produced* as f32r.
    mmdt = F32R if mm_mode == "f32r" else F32

    with tile.TileContext(nc) as tc, ExitStack() as ctx:
        const = ctx.enter_context(tc.tile_pool(name="const", bufs=1))
        big = ctx.enter_context(tc.tile_pool(name="big", bufs=tn["bufs_xy"]))
        bigo = ctx.enter_context(tc.tile_pool(name="bigo", bufs=tn["bufs_o"]))
        hp = ctx.enter_context(tc.tile_pool(name="h", bufs=tn["bufs_h"]))
        htp = ctx.enter_context(tc.tile_pool(name="ht", bufs=tn["bufs_ht"]))
        tps = ctx.enter_context(
            tc.tile_pool(name="tps", bufs=tn["bufs_tps"], space="PSUM"))
        ups = ctx.enter_context(
            tc.tile_pool(name="ups", bufs=tn["bufs_ups"], space="PSUM"))
        o2_eng = getattr(nc, tn["o2_engine"])
        st_eng = getattr(nc, tn["store_engine"])

        # N*Wv, k-chunk c on partitions at free slice c  ->  [128, KC, D]
        wN = const.tile([TP, KC, D], mmdt)
        wf = const.tile([TP, KC, D], F32)
        nc.sync.dma_start(wf[:], wv.rearrange("(c p) d -> p c d", p=TP))
        nc.scalar.mul(wN[:], wf[:], float(N))

        identf = const.tile([TP, TP], F32)
        make_identity(nc, identf[:])
        if mmdt is F32:
            ident = identf
        else:
            ident = const.tile([TP, TP], mmdt)
            nc.scalar.copy(ident[:], identf[:])

        if with_bias:
            b2f = const.tile([1, D], F32)
            nc.sync.dma_start(b2f[:], bv[:])
            b2 = const.tile([1, D], mmdt)
            nc.scalar.mul(b2[:], b2f[:], float(2 * N))
            onesf = const.tile([1, TP], F32)
            nc.gpsimd.memset(onesf[:], 1.0)
            ones = const.tile([1, TP], mmdt)
            nc.scalar.copy(ones[:], onesf[:])

        def emit_unit(u):
            # X = g_B rows straight: flat row f = t*TP + p  (affine in p, t).
            # First f-tile loaded separately so tile-0 compute starts after
            # ~0.5MB instead of the unit's full 4.2MB (trims the ramp).
            X = big.tile([TP, NT, D], F32, tag="X")
            xsrc = g_in[u, 0].rearrange("(t i2) b d -> (i2 b) t d", i2=I2)
            nc.sync.dma_start(X[:, 0:1, :], xsrc[:, 0:1, :])
            # Yp = g_B' transpose-permuted rows: Yp[(i,j)] = Y[j, i].
            # One DMA per 32-partition range (SBUF DMA APs need a pure
            # partition dim) with a strided DRAM column slice.
            Yp = big.tile([TP, NT, D], F32, tag="Yp")
            for i2 in range(I2):
                nc.sync.dma_start(Yp[i2 * W:(i2 + 1) * W, 0:1, :],
                                  g_in[u, 1][:, i2:W:I2, :][:, 0:1, :])
            nc.sync.dma_start(X[:, 1:NT, :], xsrc[:, 1:NT, :])
            for i2 in range(I2):
                nc.sync.dma_start(Yp[i2 * W:(i2 + 1) * W, 1:NT, :],
                                  g_in[u, 1][:, i2:W:I2, :][:, 1:NT, :])

            O1 = bigo.tile([TP, NT, D], F32, tag="O1")
            O2 = bigo.tile([TP, NT, D], F32, tag="O2")

            for t in range(NT):
                h = hp.tile([TP, D], mmdt)
                nc.vector.tensor_add(h[:], X[:, t, :], Yp[:, t, :])

                # hT chunks: [k-in-chunk (part), f (free slice c)]
                hT_ps = tps.tile([TP, D], mmdt)
                for c in range(KC):
                    nc.tensor.transpose(hT_ps[:, bass.ts(c, TP)],
                                        h[:, bass.ts(c, TP)],
                                        ident[:])
                hT = htp.tile([TP, D], mmdt)
                nc.scalar.copy(hT[:], hT_ps[:])

                u_ps = ups.tile([TP, D], F32)
                for c in range(KC):
                    nc.tensor.matmul(u_ps[:], hT[:, bass.ts(c, TP)],
                                     wN[:, c, :],
                                     start=(c == 0),
                                     stop=(c == KC - 1 and not with_bias))
                if with_bias:
                    # rank-1: adds 2N*bv to every output row of this tile
                    nc.tensor.matmul(u_ps[:], ones[:, :], b2[:, :],
                                     start=False, stop=True)

                nc.vector.tensor_add(O1[:, t, :], u_ps[:], X[:, t, :])
                o2_eng.tensor_add(O2[:, t, :], u_ps[:], Yp[:, t, :])

            st_eng.dma_start(
                out[u, 0].rearrange("(t i2) b d -> (i2 b) t d", i2=I2), O1[:])
            for i2 in range(I2):
                st_eng.dma_start(out[u, 1][:, i2:W:I2, :],
                                 O2[i2 * W:(i2 + 1) * W, :, :])

        if repeat > 1:
            with tc.For_i(0, repeat, 1):
                for u in range(n_units):
                    emit_unit(u)
        else:
            for u in range(n_units):
                emit_unit(u)

    nc.compile()
    return nc


_BUILD_CACHE = {}


def _get_program(n_units, with_bias, mm_mode, split_dma=True, repeat=1,
                 tune=None):
    key = (n_units, with_bias, mm_mode, split_dma, repeat,
           tuple(sorted((tune or {}).items())))
    if key not in _BUILD_CACHE:
        _BUILD_CACHE[key] = _build(n_units, with_bias, mm_mode, split_dma,
                                   repeat, tune)
    return _BUILD_CACHE[key]


def _shard(g, wv, bv, assignment, n_units, with_bias):
    Gb = np.ascontiguousarray(
        g.reshape(GB, W, GB, W, D).transpose(0, 2, 1, 3, 4))
    in_maps = []
    for units in assignment:
        gi = np.zeros((n_units, 2, W, W, D), np.float32)
        for k, unit in enumerate(units):
            if unit is None:
                continue
            A, B = unit
            gi[k, 0] = Gb[A]
            gi[k, 1] = Gb[B]
        m = {"g_in": gi, "wv": wv}
        if with_bias:
            m["bv"] = bv.reshape(1, D)
        in_maps.append(m)
    return in_maps


def _unshard(per_core_outs, assignment):
    Ob = np.empty((GB, GB, W, W, D), np.float32)
    for c, units in enumerate(assignment):
        ob = per_core_outs[c]["out_blocks"]
        for k, unit in enumerate(units):
            if unit is None:
                continue
            A, B = unit
            Ob[A] = ob[k, 0]
            if A != B:
                Ob[B] = ob[k, 1]
    return np.ascontiguousarray(
        Ob.transpose(0, 2, 1, 3, 4)).reshape(N, N, D)


def _unit_math_numpy(gi, wv, bv):
    """Numpy model of one core's device program (for self-tests)."""
    n_units = gi.shape[0]
    ob = np.zeros_like(gi)
    wN = wv * np.float32(N)
    b2 = bv * np.float32(2 * N)
    for k in range(n_units):
        X = gi[k, 0].reshape(W * W, D)
        Yp = gi[k, 1].transpose(1, 0, 2).reshape(W * W, D)
        h = X + Yp
        u = h @ wN + b2
        ob[k, 0] = (u + X).reshape(W, W, D)
        ob[k, 1] = (u + Yp).reshape(W, W, D).transpose(1, 0, 2)
    return ob


def kernel(g, Wq_w, Wq_b, Wk_w, Wk_b, Wv_w, Wv_b, _backend="hw"):
    global LAST_RESULTS
    g = np.ascontiguousarray(np.asarray(g, np.float32))
    wv = np.ascontiguousarray(np.asarray(Wv_w, np.float32))
    bv = np.ascontiguousarray(np.asarray(Wv_b, np.float32))
    with_bias = bool(np.any(bv))

    assignment, n_units = _assignment()
    in_maps = _shard(g, wv, bv, assignment, n_units, with_bias)

    if _backend == "numpy":
        outs = [{"out_blocks": _unit_math_numpy(m["g_in"], wv, bv)}
                for m in in_maps]
        return _unshard(outs, assignment)

    nc = _get_program(n_units, with_bias, MM_MODE)
    try:
        res = run_bass_kernel_spmd(nc, in_maps, core_ids=list(range(NCORES)))
    except ModuleNotFoundError:
        # BASS_TRACE set but the axon NTFF hook module isn't present in this
        # image -- retry without tracing.
        os.environ["BASS_NEVER_TRACE"] = "1"
        res = run_bass_kernel_spmd(nc, in_maps, core_ids=list(range(NCORES)))
    LAST_RESULTS = res
    return _unshard(res.results, assignment)


# revision 50
# speedup vs baseline: 1.0321x; 1.0321x over previous
"""Trainium2 Bass kernel for nn_AxisAttention (sparse_attention).

Math: the reference applies softmax over a size-1 axis, so every attention
weight is exactly 1.0 and the module collapses algebraically:

    v       = g @ Wv + bv                      # [N, N, D]
    row_att = N * v.transpose(1, 0, 2)         # sum_i of i-independent rows
    col_att = N * v
    out     = g + N*(v + v^T) + ...            # ^T swaps the first two axes
            = g + N*((g + g^T) @ Wv) + 2*N*bv

So one matmul over h = g + g^T suffices; q/k are dead code.

Sharding: the (i, j) grid is split into 32x32 blocks (12x12 of them).
A block B=(bi,bj) is paired with its transpose partner B'=(bj,bi).  With
h_B = g_B + g_B'^T(local) and u_B = h_B @ (N*Wv):

    out_B  = g_B  + u_B  (+ 2N*bv)
    out_B' = g_B' + u_B^T(local) (+ 2N*bv)      since h_B' = h_B^T(local)

so one matmul pass produces BOTH output blocks -> half the FLOPs and every
g/out byte crosses HBM exactly once.  66 pair-units + 12 diagonal units
(+2 dummies) = 80 units, 10 per core on 8 cores -- a uniform SPMD program.

On-device per unit: DMA X=g_B (straight rows) and Yp=g_B' (transpose-permuted
rows, contiguous 2KB runs), DVE h=X+Yp, PE-transpose h tiles (fp32 can't DMA
-transpose), matmul hT-tiles (stationary) against N*Wv (moving), DVE residual
adds, DMA out (straight + permuted APs).
"""

import os
from contextlib import ExitStack

import numpy as np

import concourse.bass as bass
import concourse.bacc as bacc
import concourse.mybir as mybir
import concourse.tile as tile
from concourse.bass_utils import run_bass_kernel_spmd
from concourse.masks import make_identity

# Problem constants (hardcoded per the harness contract).
N = 384          # grid side
D = 512          # feature dim (= contraction dim of Wv)
W = 32           # block side
GB = N // W      # 12 blocks per grid side
NCORES = 8
TP = 128         # SBUF/PSUM partitions per tile
I2 = TP // W     # 4 block-rows per 128-partition tile
NT = (W * W) // TP   # 8 f-tiles per block (f = i*W + j)
KC = D // TP     # 4 contraction chunks

F32 = mybir.dt.float32
F32R = mybir.dt.float32r

# "f32"  -> exact fp32 matmuls (4 PE passes per row, safest numerics)
# "f32r" -> fp32 data truncated to ~fp22 in the PE (1 pass, ~4x faster,
#           rel err ~1e-4); numerically validated against the reference.
MM_MODE = os.environ.get("AXATTN_MM_MODE", "f32")

LAST_RESULTS = None  # BassKernelResults of the most recent run (for test.py)


def _assignment():
    """80 uniform units over 8 cores: unit = (A, B) block-index pair or None."""
    pairs = [((a, b), (b, a)) for a in range(GB) for b in range(a + 1, GB)]
    diags = [((m, m), (m, m)) for m in range(GB)]
    units = pairs + diags                       # 66 + 12 = 78
    per_core = -(-len(units) // NCORES)         # 10
    units += [None] * (NCORES * per_core - len(units))
    return [units[c * per_core:(c + 1) * per_core] for c in range(NCORES)], per_core


DEFAULT_TUNE = {
    "bufs_xy": 3,     # X/Yp input staging buffers
    "bufs_o": 2,      # O1/O2 output staging buffers
    "bufs_h": 3,
    "bufs_ht": 3,
    "bufs_tps": 3,    # transpose PSUM banks
    "bufs_ups": 2,    # matmul-accum PSUM banks
    "o2_engine": "vector",  # engine for the O2 residual add
    "store_engine": "gpsimd",  # out-DMA queue; separate from the load queue
                               # (SP) to avoid head-of-line blocking: stores
                               # wait on compute and would stall later
                               # prefetch loads issued on the same sequencer
}


def _build(n_units: int, with_bias: bool, mm_mode: str, split_dma: bool = True,
           repeat: int = 1, tune: dict | None = None):
    """Build the per-core Bass/Tile program (same program on all 8 cores).

    repeat > 1 wraps the whole unit loop in a device-side For_i that redoes
    the identical work `repeat` times (idempotent) — used only for timing:
    slope between two repeat values isolates pure device time from RPC.
    """
    tn = dict(DEFAULT_TUNE)
    if tune:
        tn.update(tune)
    nc = bacc.Bacc(trn_type="TRN2", target_bir_lowering=False, debug=False)

    g_in = nc.dram_tensor("g_in", [n_units, 2, W, W, D], F32,
                          kind="ExternalInput").ap()
    wv = nc.dram_tensor("wv", [D, D], F32, kind="ExternalInput").ap()
    out = nc.dram_tensor("out_blocks", [n_units, 2, W, W, D], F32,
                         kind="ExternalOutput").ap()
    if with_bias:
        bv = nc.dram_tensor("bv", [1, D], F32, kind="ExternalInput").ap()

    # float32r: PE reads fp32 bits truncated to ~fp22 and runs 1 pass/row
    # instead of fp32's 4 (4x matmul throughput, ~1.6e-4 rel err measured).
    # The BIR verifier requires every PE input to be *produced* as f32r.
    mmdt = F32R if mm_mode == "f32r" else F32

    with tile.TileContext(nc) as tc, ExitStack() as ctx:
        const = ctx.enter_context(tc.tile_pool(name="const", bufs=1))
        big = ctx.enter_context(tc.tile_pool(name="big", bufs=tn["bufs_xy"]))
        bigo = ctx.enter_context(tc.tile_pool(name="bigo", bufs=tn["bufs_o"]))
        hp = ctx.enter_context(tc.tile_pool(name="h", bufs=tn["bufs_h"]))
        htp = ctx.enter_context(tc.tile_pool(name="ht", bufs=tn["bufs_ht"]))
        tps = ctx.enter_context(
            tc.tile_pool(name="tps", bufs=tn["bufs_tps"], space="PSUM"))
        ups = ctx.enter_context(
            tc.tile_pool(name="ups", bufs=tn["bufs_ups"], space="PSUM"))
        o2_eng = getattr(nc, tn["o2_engine"])
        st_eng = getattr(nc, tn["store_engine"])

        # N*Wv, k-chunk c on partitions at free slice c  ->  [128, KC, D]
        wN = const.tile([TP, KC, D], mmdt)
        wf = const.tile([TP, KC, D], F32)
        nc.sync.dma_start(wf[:], wv.rearrange("(c p) d -> p c d", p=TP))
        nc.scalar.mul(wN[:], wf[:], float(N))

        identf = const.tile([TP, TP], F32)
        make_identity(nc, identf[:])
        if mmdt is F32:
            ident = identf
        else:
            ident = const.tile([TP, TP], mmdt)
            nc.scalar.copy(ident[:], identf[:])

        if with_bias:
            b2f = const.tile([1, D], F32)
            nc.sync.dma_start(b2f[:], bv[:])
            b2 = const.tile([1, D], mmdt)
            nc.scalar.mul(b2[:], b2f[:], float(2 * N))
            onesf = const.tile([1, TP], F32)
            nc.gpsimd.memset(onesf[:], 1.0)
            ones = const.tile([1, TP], mmdt)
            nc.scalar.copy(ones[:], onesf[:])

        def emit_unit(u):
            # X = g_B rows straight: flat row f = t*TP + p  (affine in p, t).
            # Yp = g_B' with the (i,j)->(j,i) permutation pre-applied during
            # the host gather (which strided-copies every byte anyway), so
            # BOTH loads are fully-contiguous 2MB DMAs -- the device-side
            # permuted AP moved data in 2KB descriptor chunks at reduced DMA
            # efficiency.  First f-tile loaded separately so tile-0 compute
            # starts after ~0.5MB instead of the unit's full 4.2MB.
            X = big.tile([TP, NT, D], F32, tag="X")
            Yp = big.tile([TP, NT, D], F32, tag="Yp")
            xsrc = g_in[u, 0].rearrange("(t i2) b d -> (i2 b) t d", i2=I2)
            ysrc = g_in[u, 1].rearrange("(t i2) b d -> (i2 b) t d", i2=I2)
            nc.sync.dma_start(X[:, 0:1, :], xsrc[:, 0:1, :])
            nc.sync.dma_start(Yp[:, 0:1, :], ysrc[:, 0:1, :])
            nc.sync.dma_start(X[:, 1:NT, :], xsrc[:, 1:NT, :])
            nc.sync.dma_start(Yp[:, 1:NT, :], ysrc[:, 1:NT, :])

            O1 = bigo.tile([TP, NT, D], F32, tag="O1")
            O2 = bigo.tile([TP, NT, D], F32, tag="O2")

            for t in range(NT):
                h = hp.tile([TP, D], mmdt)
                nc.vector.tensor_add(h[:], X[:, t, :], Yp[:, t, :])

                # hT chunks: [k-in-chunk (part), f (free slice c)]
                hT_ps = tps.tile([TP, D], mmdt)
                for c in range(KC):
                    nc.tensor.transpose(hT_ps[:, bass.ts(c, TP)],
                                        h[:, bass.ts(c, TP)],
                                        ident[:])
                hT = htp.tile([TP, D], mmdt)
                nc.scalar.copy(hT[:], hT_ps[:])

                u_ps = ups.tile([TP, D], F32)
                for c in range(KC):
                    nc.tensor.matmul(u_ps[:], hT[:, bass.ts(c, TP)],
                                     wN[:, c, :],
                                     start=(c == 0),
                                     stop=(c == KC - 1 and not with_bias))
                if with_bias:
                    # rank-1: adds 2N*bv to every output row of this tile
                    nc.tensor.matmul(u_ps[:], ones[:, :], b2[:, :],
                                     start=False, stop=True)

                nc.vector.tensor_add(O1[:, t, :], u_ps[:], X[:, t, :])
                o2_eng.tensor_add(O2[:, t, :], u_ps[:], Yp[:, t, :])

            # O2 written in straight (i,j) row order; the host unshard
            # applies the inverse (j,i) permutation when placing block B'.
            st_eng.dma_start(
                out[u, 0].rearrange("(t i2) b d -> (i2 b) t d", i2=I2), O1[:])
            st_eng.dma_start(
                out[u, 1].rearrange("(t i2) b d -> (i2 b) t d", i2=I2), O2[:])

        if repeat > 1:
            with tc.For_i(0, repeat, 1):
                for u in range(n_units):
                    emit_unit(u)
        else:
            for u in range(n_units):
                emit_unit(u)

    nc.compile()
    return nc


_BUILD_CACHE = {}


def _get_program(n_units, with_bias, mm_mode, split_dma=True, repeat=1,
                 tune=None):
    key = (n_units, with_bias, mm_mode, split_dma, repeat,
           tuple(sorted((tune or {}).items())))
    if key not in _BUILD_CACHE:
        _BUILD_CACHE[key] = _build(n_units, with_bias, mm_mode, split_dma,
                                   repeat, tune)
    return _BUILD_CACHE[key]


def _shard(g, wv, bv, assignment, n_units, with_bias):
    Gb = np.ascontiguousarray(
        g.reshape(GB, W, GB, W, D).transpose(0, 2, 1, 3, 4))
    in_maps = []
    for units in assignment:
        gi = np.zeros((n_units, 2, W, W, D), np.float32)
        for k, unit in enumerate(units):
            if unit is None:
                continue
            A, B = unit
            gi[k, 0] = Gb[A]
            gi[k, 1] = Gb[B].transpose(1, 0, 2)  # pre-permuted: Yp[i,j]=Y[j,i]
        m = {"g_in": gi, "wv": wv}
        if with_bias:
            m["bv"] = bv.reshape(1, D)
        in_maps.append(m)
    return in_maps


def _unshard(per_core_outs, assignment):
    Ob = np.empty((GB, GB, W, W, D), np.float32)
    for c, units in enumerate(assignment):
        ob = per_core_outs[c]["out_blocks"]
        for k, unit in enumerate(units):
            if unit is None:
                continue
            A, B = unit
            Ob[A] = ob[k, 0]
            if A != B:
                # device wrote O2 in (i,j) order; block B' wants (j,i)
                Ob[B] = ob[k, 1].transpose(1, 0, 2)
    return np.ascontiguousarray(
        Ob.transpose(0, 2, 1, 3, 4)).reshape(N, N, D)


def _unit_math_numpy(gi, wv, bv):
    """Numpy model of one core's device program (for self-tests)."""
    n_units = gi.shape[0]
    ob = np.zeros_like(gi)
    wN = wv * np.float32(N)
    b2 = bv * np.float32(2 * N)
    for k in range(n_units):
        X = gi[k, 0].reshape(W * W, D)
        Yp = gi[k, 1].reshape(W * W, D)  # host-permuted on input
        h = X + Yp
        u = h @ wN + b2
        ob[k, 0] = (u + X).reshape(W, W, D)
        ob[k, 1] = (u + Yp).reshape(W, W, D)  # host un-permutes on unshard
    return ob


def kernel(g, Wq_w, Wq_b, Wk_w, Wk_b, Wv_w, Wv_b, _backend="hw"):
    global LAST_RESULTS
    g = np.ascontiguousarray(np.asarray(g, np.float32))
    wv = np.ascontiguousarray(np.asarray(Wv_w, np.float32))
    bv = np.ascontiguousarray(np.asarray(Wv_b, np.float32))
    with_bias = bool(np.any(bv))

    assignment, n_units = _assignment()
    in_maps = _shard(g, wv, bv, assignment, n_units, with_bias)

    if _backend == "numpy":
        outs = [{"out_blocks": _unit_math_numpy(m["g_in"], wv, bv)}
                for m in in_maps]
        return _unshard(outs, assignment)

    nc = _get_program(n_units, with_bias, MM_MODE)
    try:
        res = run_bass_kernel_spmd(nc, in_maps, core_ids=list(range(NCORES)))
    except ModuleNotFoundError:
        # BASS_TRACE set but the axon NTFF hook module isn't present in this
        # image -- retry without tracing.
        os.environ["BASS_NEVER_TRACE"] = "1"
        res = run_bass_kernel_spmd(nc, in_maps, core_ids=list(range(NCORES)))
    LAST_RESULTS = res
    return _unshard(res.results, assignment)
